# revision 38
# baseline (speedup 1.0000x reference)
"""Trainium2 Bass kernel for nn_AttentionHead (sparse attention, 8 cores).

Reference computation (per batch b):
    q = x_q @ wq^T ; k = x_k @ wk^T ; v = x_v @ wv^T          # [S, H]
    s = (q @ k^T) / sqrt(H)                                    # [S, S]
    s = where(mask == 0, 0, s)       # multiplicative 0/1 mask BEFORE softmax
    p = softmax(s, axis=-1)          # masked entries contribute exp(0)=1
    out = p @ v                                                # [S, H]

Sharding: 8 cores; core c -> batch c//2, query rows (c%2)*2048 ... +2048.
Each core computes k/v for its whole batch (duplicated within the pair),
so there are no collectives.

Design (HW-measured 215.2-215.6us vs the 317.5us starting baseline;
PE busy ~186us vs ~180us MAC-count floor, start/tail ~26us fixed):
  * Scores are computed TRANSPOSED: for each 128-row sk chunk,
    sT[sk128, sq512] = K-chunk @ Q^T via matmul(lhsT=kT slice, rhs=qT block).
    After mask-mult + exp, pT[sk, sq] is already the lhsT layout the PV
    matmul needs -> the 512 PE transposes and 128 PSUM->SBUF bounce copies
    of the old formulation are gone.
  * Every matmul is bf16: HW shows fp32r matmuls at free-dim 256 run ~4x
    slower (390ns vs 107ns), and bf16 enables fast weight load. Accuracy
    emulated on CPU: rel err ~3.5e-3 (budget 2e-2); HW measures 3.44e-3.
  * Softmax denominator comes free from a ones-column appended to V
    (out[:, 256] = rowsum(P)) instead of ACT accumulator reads.
  * Mask is pre-cast to fp8e4m3 (0.0/1.0 exact) and pre-blocked on the host
    so each per-sqb slab is one fully-contiguous DMA; slab 0 is emitted
    early so it beats phase-B start.
  * q is projected FIRST, and the phase-B working pools are declared
    outside the phase-A pool scope: if they alias the x tiles, the first
    phase-B ops inherit WAR waits on the last x DMAs (~9us stall).
  * Phase B is software-pipelined by one chunk so the PE stream interleaves
    scores(c+1) with PV(c) and never waits on the DVE/ACT chain.

Rejected by measurement: pair-sharing k/v projections via 2-core AllGather
(tried twice, incl. with exchange DMAs emitted at the front of the stream:
the CC engine's first mesh phase does not even begin until ~90us and the
gather lands ~148us -- huge fixed latency on this fleet); fp8 PV matmul
(rel err 3.4e-2 > 2e-2 budget); reordering early DMA emission (the slow
~130GB/s early stream makes shuffles regress).
"""

import numpy as np
import ml_dtypes

import concourse.bass as bass
import concourse.mybir as mybir
import concourse.tile as tile
from concourse import bacc
from concourse.bass_utils import run_bass_kernel_spmd

F32 = mybir.dt.float32
BF16 = mybir.dt.bfloat16
FP8 = mybir.dt.float8e4

# Full-problem constants
B, S, DV, H = 4, 4096, 1024, 256
N_CORES = 8
CORES_PER_BATCH = N_CORES // B
SQL = S // CORES_PER_BATCH  # query rows per core

P = 128
SKB = 512                    # block width (sq blocks and x blocks)
DC = DV // P                 # contraction chunks (8)
NSKB = S // SKB              # sk blocks of 512 (8)
NKC = S // P                 # sk chunks of 128 (32)
NSQB = SQL // SKB            # sq blocks of 512 (4)
HC = H // P                  # h chunks (2)
HP1 = H + 1                  # v columns incl. ones column (257)


def build_attention_nc(num_devices=1):
    nc = bacc.Bacc("TRN2", target_bir_lowering=False, debug=False,
                   num_devices=num_devices)

    # Host-blocked layouts: each leading index is one fully-contiguous DMA.
    x_q = nc.dram_tensor("x_q", [NSQB, P, DC, SKB], BF16, kind="ExternalInput").ap()
    x_k = nc.dram_tensor("x_k", [NSKB, P, DC, SKB], BF16, kind="ExternalInput").ap()
    x_v = nc.dram_tensor("x_v", [NSKB, P, DC, SKB], BF16, kind="ExternalInput").ap()
    mT = nc.dram_tensor("mT", [NSQB, P, NKC, SKB], FP8, kind="ExternalInput").ap()
    wq = nc.dram_tensor("wq", [P, DC, H], BF16, kind="ExternalInput").ap()
    wk = nc.dram_tensor("wk", [P, DC, H], BF16, kind="ExternalInput").ap()
    wv = nc.dram_tensor("wv", [P, DC, H], BF16, kind="ExternalInput").ap()
    out = nc.dram_tensor("out", [SQL, H], F32, kind="ExternalOutput").ap()

    scale = 1.0 / 16.0  # 1/sqrt(H)

    with tile.TileContext(nc) as tc:
        with (
            tc.tile_pool(name="weights", bufs=3) as w_pool,
            tc.tile_pool(name="maskp", bufs=3) as mask_pool,
            tc.tile_pool(name="kT", bufs=HC * NSKB) as kT_pool,
            tc.tile_pool(name="qT", bufs=HC * NSQB) as qT_pool,
            tc.tile_pool(name="vsb", bufs=NKC) as v_pool,
            # phase-B working pools live OUTSIDE the phase-A scope so they do
            # not alias the x tiles: aliasing makes the first phase-B ops
            # inherit WAR waits on the last x DMAs (~9us stall at the
            # phase boundary).
            tc.tile_pool(name="smp", bufs=4) as sm_pool,
            tc.tile_pool(name="pp", bufs=5) as p_pool,
            tc.tile_pool(name="osbp", bufs=2) as o_sb_pool,
            tc.tile_pool(name="ooutp", bufs=2) as o_out_pool,
            tc.tile_pool(name="denp", bufs=2) as den_pool,
        ):
            # ---- weights ----
            w_sb = {}
            for name, wT in (("q", wq), ("k", wk), ("v", wv)):
                t = w_pool.tile([P, DC, H], BF16, tag=f"w_{name}")
                nc.sync.dma_start(out=t[:], in_=wT)
                w_sb[name] = t

            kT_sb = [[None] * NSKB for _ in range(HC)]
            qT_sb = [[None] * NSQB for _ in range(HC)]
            v_sb = [None] * NKC

            # ---- phase A: projections (all-bf16 matmuls), q first ----
            m_sb = [None] * NSQB
            with (
                tc.tile_pool(name="xkp", bufs=2) as xk_pool,
                tc.tile_pool(name="xvp", bufs=2) as xv_pool,
                tc.tile_pool(name="xqp", bufs=2) as xq_pool,
                tc.tile_pool(name="psA", bufs=2, space="PSUM") as psA,
                tc.tile_pool(name="psV", bufs=2, space="PSUM") as psV,
            ):
                # HAM warmup: the first x DMA takes ~11us to land (slow early
                # stream) and an idle PE re-throttles to 1.2GHz after ~3.4us,
                # making the first ~8 real matmuls run at half rate. Burn the
                # wait on dummy matmuls over scratch data to hold K=8/8.
                scratch = w_pool.tile([P, SKB], BF16, tag="warm")
                nc.gpsimd.memset(scratch[:], 0.0)
                ps_w = psV.tile([P, SKB], F32, tag="psW")
                NWARM = 30
                for i in range(NWARM):
                    nc.tensor.matmul(
                        ps_w[:], scratch[:, 0:P], scratch[:],
                        start=(i == 0), stop=(i == NWARM - 1))

                # sqb 0 is split into half-width tiles so its first matmuls
                # only wait on 0.5MB of x_q instead of 1MB.
                xq0 = []
                for hx in range(2):
                    xt = xq_pool.tile([P, DC, SKB // 2], BF16, tag="xq")
                    nc.sync.dma_start(
                        out=xt[:],
                        in_=x_q[0][:, :, hx * (SKB // 2):(hx + 1) * (SKB // 2)])
                    xq0.append(xt)
                qt0 = []
                for hc in range(HC):
                    t = qT_pool.tile([P, SKB], BF16, tag="qT")
                    qT_sb[hc][0] = t
                    qt0.append(t)
                for hcx in range(HC * 2):
                    hx, hc = divmod(hcx, 2)
                    ps = psA.tile([P, SKB // 2], F32, tag="psA")
                    for dc in range(DC):
                        nc.tensor.matmul(
                            ps[:],
                            w_sb["q"][:, dc, hc * P:(hc + 1) * P],
                            xq0[hx][:, dc, :],
                            start=(dc == 0), stop=(dc == DC - 1))
                    nc.scalar.copy(
                        qt0[hc][:, hx * (SKB // 2):(hx + 1) * (SKB // 2)],
                        ps[:])

                for sqb in range(1, NSQB):
                    xqt = xq_pool.tile([P, DC, SKB], BF16, tag="xq")
                    nc.sync.dma_start(out=xqt[:], in_=x_q[sqb])
                    for hc in range(HC):
                        ps = psA.tile([P, SKB], F32, tag="psA")
                        for dc in range(DC):
                            nc.tensor.matmul(
                                ps[:],
                                w_sb["q"][:, dc, hc * P:(hc + 1) * P],
                                xqt[:, dc, :],
                                start=(dc == 0), stop=(dc == DC - 1))
                        t = qT_pool.tile([P, SKB], BF16, tag="qT")
                        nc.scalar.copy(t[:], ps[:])
                        qT_sb[hc][sqb] = t

                for skb in range(NSKB):
                    xkt = xk_pool.tile([P, DC, SKB], BF16, tag="xk")
                    nc.sync.dma_start(out=xkt[:], in_=x_k[skb])
                    if skb == 1:
                        # mask slab 0 is needed right at phase-B start; emit
                        # its DMA early so it stripes into the stream before
                        # the tail of the x loads.
                        t = mask_pool.tile([P, NKC, SKB], FP8, tag="mask")
                        nc.sync.dma_start(out=t[:], in_=mT[0])
                        m_sb[0] = t
                    for hc in range(HC):
                        ps = psA.tile([P, SKB], F32, tag="psA")
                        for dc in range(DC):
                            nc.tensor.matmul(
                                ps[:],
                                w_sb["k"][:, dc, hc * P:(hc + 1) * P],
                                xkt[:, dc, :],
                                start=(dc == 0), stop=(dc == DC - 1))
                        t = kT_pool.tile([P, SKB], BF16, tag="kT")
                        nc.scalar.copy(t[:], ps[:])
                        kT_sb[hc][skb] = t

                    xvt = xv_pool.tile([P, DC, SKB], BF16, tag="xv")
                    nc.sync.dma_start(out=xvt[:], in_=x_v[skb])
                    for j in range(SKB // P):
                        kc = skb * (SKB // P) + j
                        ps = psV.tile([P, H], F32, tag="psV")
                        for dc in range(DC):
                            nc.tensor.matmul(
                                ps[:],
                                xvt[:, dc, j * P:(j + 1) * P],
                                w_sb["v"][:, dc, :],
                                start=(dc == 0), stop=(dc == DC - 1))
                        t = v_pool.tile([P, HP1], BF16, tag="v")
                        nc.vector.memset(t[:, H:HP1], 1.0)
                        nc.vector.tensor_copy(t[:, 0:H], ps[:])
                        v_sb[kc] = t

            # ---- remaining mask slabs (needed at ~+28us intervals) ----
            for sqb in range(1, NSQB):
                t = mask_pool.tile([P, NKC, SKB], FP8, tag="mask")
                nc.sync.dma_start(out=t[:], in_=mT[sqb])
                m_sb[sqb] = t

            # ---- phase B: attention, sq-block-major, sk-chunk pipeline ----
            with (
                tc.tile_pool(name="spsum", bufs=3, space="PSUM") as s_psum,
                tc.tile_pool(name="opsum", bufs=5, space="PSUM") as o_psum,
            ):
                for sqb in range(NSQB):
                    o_ps = []
                    for j2 in range(SKB // P):
                        o_t = o_psum.tile([P, HP1], F32, tag="opsum")
                        o_ps.append(o_t)

                    prev = None
                    for kc in range(NKC + 1):
                        if kc < NKC:
                            skb, j = divmod(kc, SKB // P)
                            sp = s_psum.tile([P, SKB], F32, tag="spsum")
                            nc.tensor.matmul(
                                sp[:],
                                kT_sb[0][skb][:, j * P:(j + 1) * P],
                                qT_sb[0][sqb][:],
                                start=True, stop=False)
                            nc.tensor.matmul(
                                sp[:],
                                kT_sb[1][skb][:, j * P:(j + 1) * P],
                                qT_sb[1][sqb][:],
                                start=False, stop=True)
                        else:
                            sp = None
                        if prev is not None:
                            pkc, psp = prev
                            sm = sm_pool.tile([P, SKB], F32, tag="sm")
                            nc.vector.tensor_tensor(
                                sm[:], psp[:], m_sb[sqb][:, pkc, :],
                                op=mybir.AluOpType.mult)
                            pT = p_pool.tile([P, SKB], BF16, tag="p")
                            nc.scalar.activation(
                                pT[:], sm[:],
                                mybir.ActivationFunctionType.Exp,
                                scale=scale)
                            for j2 in range(SKB // P):
                                nc.tensor.matmul(
                                    o_ps[j2][:],
                                    pT[:, j2 * P:(j2 + 1) * P],
                                    v_sb[pkc][:],
                                    start=(pkc == 0),
                                    stop=(pkc == NKC - 1))
                        prev = (kc, sp) if sp is not None else None

                    # epilogue: normalize by the ones-column rowsum and store
                    for j2 in range(SKB // P):
                        osb = o_sb_pool.tile([P, HP1], F32, tag="osb")
                        nc.scalar.copy(osb[:], o_ps[j2][:])
                        den = den_pool.tile([P, 1], F32, tag="den")
                        nc.vector.reciprocal(den[:], osb[:, H:HP1])
                        oout = o_out_pool.tile([P, H], F32, tag="oout")
                        nc.vector.tensor_scalar_mul(
                            oout[:], osb[:, 0:H], den[:])
                        r0 = sqb * SKB + j2 * P
                        nc.sync.dma_start(out=out[r0:r0 + P, :], in_=oout[:])

    nc.compile()
    return nc


_COMPILED = None

# test-harness knobs (ignored in normal use)
TRACE = False
LAST_RESULT = None


def _get_compiled():
    global _COMPILED
    if _COMPILED is None:
        _COMPILED = build_attention_nc(num_devices=N_CORES)
    return _COMPILED


def _block_xT(xT):
    """[DV, W] f32 -> [W//SKB, P, DC, SKB] bf16 with
    blocks[wb, p, dc, w] = xT[dc*P + p, wb*SKB + w]."""
    W = xT.shape[1]
    b = xT.reshape(DC, P, W // SKB, SKB).transpose(2, 1, 0, 3)
    return np.ascontiguousarray(b.astype(ml_dtypes.bfloat16))


def _block_w(w):
    """[H, DV] f32 -> [P, DC, H] bf16 with blocks[p, dc, h] = w[h, dc*P+p]."""
    b = w.T.reshape(DC, P, H).transpose(1, 0, 2)
    return np.ascontiguousarray(b.astype(ml_dtypes.bfloat16))


def _block_maskT(maskT):
    """[S, SQL] -> [NSQB, P, NKC, SKB] fp8 with
    blocks[sqb, p, c, w] = maskT[c*P + p, sqb*SKB + w]."""
    b = maskT.reshape(NKC, P, NSQB, SKB).transpose(2, 1, 0, 3)
    return np.ascontiguousarray(b.astype(ml_dtypes.float8_e4m3))


def kernel(x_q, x_k, x_v, mask, wq_w, wq_b, wk_w, wk_b, wv_w, wv_b):
    """Full inputs in, full output out. Shards across 8 NeuronCores."""
    nc = _get_compiled()

    x_q = np.asarray(x_q, dtype=np.float32)
    x_k = np.asarray(x_k, dtype=np.float32)
    x_v = np.asarray(x_v, dtype=np.float32)
    mask = np.asarray(mask)

    wqb = _block_w(np.asarray(wq_w, dtype=np.float32))
    wkb = _block_w(np.asarray(wk_w, dtype=np.float32))
    wvb = _block_w(np.asarray(wv_w, dtype=np.float32))

    in_maps = []
    for c in range(N_CORES):
        b, half = divmod(c, CORES_PER_BATCH)
        q0 = half * SQL
        xqT = x_q[b][q0:q0 + SQL].T            # [DV, SQL]
        xkT = x_k[b].T                         # [DV, S]
        xvT = x_v[b].T
        maskT = mask[b][q0:q0 + SQL].T         # [S(k), SQL(q)]
        in_maps.append({
            "x_q": _block_xT(xqT),
            "x_k": _block_xT(xkT),
            "x_v": _block_xT(xvT),
            "mT": _block_maskT(maskT),
            "wq": wqb,
            "wk": wkb,
            "wv": wvb,
        })

    global LAST_RESULT
    res = run_bass_kernel_spmd(nc, in_maps, core_ids=list(range(N_CORES)),
                               trace=TRACE)
    LAST_RESULT = res
    outs = res.results

    full = np.empty((B, S, H), dtype=np.float32)
    for c in range(N_CORES):
        b, half = divmod(c, CORES_PER_BATCH)
        q0 = half * SQL
        full[b, q0:q0 + SQL] = outs[c]["out"]
    return full


# revision 41
# speedup vs baseline: 1.2009x; 1.2009x over previous
"""Trainium2 Bass kernel for nn_AttentionHead (sparse attention, 8 cores).

Reference computation (per batch b):
    q = x_q @ wq^T ; k = x_k @ wk^T ; v = x_v @ wv^T          # [S, H]
    s = (q @ k^T) / sqrt(H)                                    # [S, S]
    s = where(mask == 0, 0, s)       # multiplicative 0/1 mask BEFORE softmax
    p = softmax(s, axis=-1)          # masked entries contribute exp(0)=1
    out = p @ v                                                # [S, H]

Sharding: 8 cores; core c -> batch c//2, query rows (c%2)*2048 ... +2048.
Each core computes k/v for its whole batch (duplicated within the pair),
so there are no collectives.

Design (HW-measured 215.2-215.6us vs the 317.5us starting baseline;
PE busy ~186us vs ~180us MAC-count floor, start/tail ~26us fixed):
  * Scores are computed TRANSPOSED: for each 128-row sk chunk,
    sT[sk128, sq512] = K-chunk @ Q^T via matmul(lhsT=kT slice, rhs=qT block).
    After mask-mult + exp, pT[sk, sq] is already the lhsT layout the PV
    matmul needs -> the 512 PE transposes and 128 PSUM->SBUF bounce copies
    of the old formulation are gone.
  * Every matmul is bf16: HW shows fp32r matmuls at free-dim 256 run ~4x
    slower (390ns vs 107ns), and bf16 enables fast weight load. Accuracy
    emulated on CPU: rel err ~3.5e-3 (budget 2e-2); HW measures 3.44e-3.
  * Softmax denominator comes free from a ones-column appended to V
    (out[:, 256] = rowsum(P)) instead of ACT accumulator reads.
  * Mask is pre-cast to fp8e4m3 (0.0/1.0 exact) and pre-blocked on the host
    so each per-sqb slab is one fully-contiguous DMA; slab 0 is emitted
    early so it beats phase-B start.
  * q is projected FIRST, and the phase-B working pools are declared
    outside the phase-A pool scope: if they alias the x tiles, the first
    phase-B ops inherit WAR waits on the last x DMAs (~9us stall).
  * Phase B is software-pipelined by one chunk so the PE stream interleaves
    scores(c+1) with PV(c) and never waits on the DVE/ACT chain.

Rejected by measurement: pair-sharing k/v projections via 2-core AllGather
(tried twice, incl. with exchange DMAs emitted at the front of the stream:
the CC engine's first mesh phase does not even begin until ~90us and the
gather lands ~148us -- huge fixed latency on this fleet); fp8 PV matmul
(rel err 3.4e-2 > 2e-2 budget); reordering early DMA emission (the slow
~130GB/s early stream makes shuffles regress).
"""

import numpy as np
import ml_dtypes

import concourse.bass as bass
import concourse.mybir as mybir
import concourse.tile as tile
from concourse import bacc
from concourse.bass_utils import run_bass_kernel_spmd

F32 = mybir.dt.float32
BF16 = mybir.dt.bfloat16
FP8 = mybir.dt.float8e4

# Full-problem constants
B, S, DV, H = 4, 4096, 1024, 256
N_CORES = 8
CORES_PER_BATCH = N_CORES // B
SQL = S // CORES_PER_BATCH  # query rows per core

P = 128
SKB = 512                    # block width (sq blocks and x blocks)
DC = DV // P                 # contraction chunks (8)
NSKB = S // SKB              # sk blocks of 512 (8)
NKC = S // P                 # sk chunks of 128 (32)
NSQB = SQL // SKB            # sq blocks of 512 (4)
HC = H // P                  # h chunks (2)
HP1 = H + 1                  # v columns incl. ones column (257)


def build_attention_nc(num_devices=1):
    nc = bacc.Bacc("TRN2", target_bir_lowering=False, debug=False,
                   num_devices=num_devices)

    # Host-blocked layouts: each leading index is one fully-contiguous DMA.
    x_q = nc.dram_tensor("x_q", [NSQB, P, DC, SKB], BF16, kind="ExternalInput").ap()
    x_k = nc.dram_tensor("x_k", [NSKB, P, DC, SKB], BF16, kind="ExternalInput").ap()
    x_v = nc.dram_tensor("x_v", [NSKB, P, DC, SKB], BF16, kind="ExternalInput").ap()
    mT = nc.dram_tensor("mT", [NSQB, P, NKC, SKB], FP8, kind="ExternalInput").ap()
    wq = nc.dram_tensor("wq", [P, DC, H], BF16, kind="ExternalInput").ap()
    wk = nc.dram_tensor("wk", [P, DC, H], BF16, kind="ExternalInput").ap()
    wv = nc.dram_tensor("wv", [P, DC, H], BF16, kind="ExternalInput").ap()
    out = nc.dram_tensor("out", [SQL, H], F32, kind="ExternalOutput").ap()

    scale = 1.0 / 16.0  # 1/sqrt(H)

    with tile.TileContext(nc) as tc:
        with (
            tc.tile_pool(name="weights", bufs=3) as w_pool,
            tc.tile_pool(name="maskp", bufs=3) as mask_pool,
            tc.tile_pool(name="kT", bufs=HC * NSKB) as kT_pool,
            tc.tile_pool(name="qT", bufs=HC * NSQB) as qT_pool,
            tc.tile_pool(name="vsb", bufs=NKC) as v_pool,
            # phase-B working pools live OUTSIDE the phase-A scope so they do
            # not alias the x tiles: aliasing makes the first phase-B ops
            # inherit WAR waits on the last x DMAs (~9us stall at the
            # phase boundary).
            tc.tile_pool(name="smp", bufs=4) as sm_pool,
            tc.tile_pool(name="pp", bufs=5) as p_pool,
            tc.tile_pool(name="osbp", bufs=2) as o_sb_pool,
            tc.tile_pool(name="ooutp", bufs=2) as o_out_pool,
            tc.tile_pool(name="denp", bufs=2) as den_pool,
        ):
            # ---- weights ----
            w_sb = {}
            for name, wT in (("q", wq), ("k", wk), ("v", wv)):
                t = w_pool.tile([P, DC, H], BF16, tag=f"w_{name}")
                nc.sync.dma_start(out=t[:], in_=wT)
                w_sb[name] = t

            kT_sb = [[None] * NSKB for _ in range(HC)]
            qT_sb = [[None] * NSQB for _ in range(HC)]
            v_sb = [None] * NKC

            # ---- phase A: projections (all-bf16 matmuls), q first ----
            m_sb = [None] * NSQB
            with (
                tc.tile_pool(name="xkp", bufs=2) as xk_pool,
                tc.tile_pool(name="xvp", bufs=2) as xv_pool,
                tc.tile_pool(name="xqp", bufs=2) as xq_pool,
                tc.tile_pool(name="psA", bufs=2, space="PSUM") as psA,
                tc.tile_pool(name="psV", bufs=2, space="PSUM") as psV,
            ):
                for sqb in range(NSQB):
                    xqt = xq_pool.tile([P, DC, SKB], BF16, tag="xq")
                    nc.sync.dma_start(out=xqt[:], in_=x_q[sqb])
                    for hc in range(HC):
                        ps = psA.tile([P, SKB], F32, tag="psA")
                        for dc in range(DC):
                            nc.tensor.matmul(
                                ps[:],
                                w_sb["q"][:, dc, hc * P:(hc + 1) * P],
                                xqt[:, dc, :],
                                start=(dc == 0), stop=(dc == DC - 1))
                        t = qT_pool.tile([P, SKB], BF16, tag="qT")
                        nc.scalar.copy(t[:], ps[:])
                        qT_sb[hc][sqb] = t

                for skb in range(NSKB):
                    xkt = xk_pool.tile([P, DC, SKB], BF16, tag="xk")
                    nc.sync.dma_start(out=xkt[:], in_=x_k[skb])
                    if skb == 1:
                        # mask slab 0 is needed right at phase-B start; emit
                        # its DMA early so it stripes into the stream before
                        # the tail of the x loads.
                        t = mask_pool.tile([P, NKC, SKB], FP8, tag="mask")
                        nc.sync.dma_start(out=t[:], in_=mT[0])
                        m_sb[0] = t
                    for hc in range(HC):
                        ps = psA.tile([P, SKB], F32, tag="psA")
                        for dc in range(DC):
                            nc.tensor.matmul(
                                ps[:],
                                w_sb["k"][:, dc, hc * P:(hc + 1) * P],
                                xkt[:, dc, :],
                                start=(dc == 0), stop=(dc == DC - 1))
                        t = kT_pool.tile([P, SKB], BF16, tag="kT")
                        nc.scalar.copy(t[:], ps[:])
                        kT_sb[hc][skb] = t

                    xvt = xv_pool.tile([P, DC, SKB], BF16, tag="xv")
                    nc.sync.dma_start(out=xvt[:], in_=x_v[skb])
                    for j in range(SKB // P):
                        kc = skb * (SKB // P) + j
                        ps = psV.tile([P, H], F32, tag="psV")
                        for dc in range(DC):
                            nc.tensor.matmul(
                                ps[:],
                                xvt[:, dc, j * P:(j + 1) * P],
                                w_sb["v"][:, dc, :],
                                start=(dc == 0), stop=(dc == DC - 1))
                        t = v_pool.tile([P, HP1], BF16, tag="v")
                        nc.vector.memset(t[:, H:HP1], 1.0)
                        nc.vector.tensor_copy(t[:, 0:H], ps[:])
                        v_sb[kc] = t

            # ---- remaining mask slabs (needed at ~+28us intervals) ----
            for sqb in range(1, NSQB):
                t = mask_pool.tile([P, NKC, SKB], FP8, tag="mask")
                nc.sync.dma_start(out=t[:], in_=mT[sqb])
                m_sb[sqb] = t

            # ---- phase B: attention, sq-block-major, sk-chunk pipeline ----
            with (
                tc.tile_pool(name="spsum", bufs=3, space="PSUM") as s_psum,
                tc.tile_pool(name="opsum", bufs=5, space="PSUM") as o_psum,
            ):
                for sqb in range(NSQB):
                    o_ps = []
                    for j2 in range(SKB // P):
                        o_t = o_psum.tile([P, HP1], F32, tag="opsum")
                        o_ps.append(o_t)

                    prev = None
                    for kc in range(NKC + 1):
                        if kc < NKC:
                            skb, j = divmod(kc, SKB // P)
                            sp = s_psum.tile([P, SKB], F32, tag="spsum")
                            nc.tensor.matmul(
                                sp[:],
                                kT_sb[0][skb][:, j * P:(j + 1) * P],
                                qT_sb[0][sqb][:],
                                start=True, stop=False)
                            nc.tensor.matmul(
                                sp[:],
                                kT_sb[1][skb][:, j * P:(j + 1) * P],
                                qT_sb[1][sqb][:],
                                start=False, stop=True)
                        else:
                            sp = None
                        if prev is not None:
                            pkc, psp = prev
                            sm = sm_pool.tile([P, SKB], F32, tag="sm")
                            nc.vector.tensor_tensor(
                                sm[:], psp[:], m_sb[sqb][:, pkc, :],
                                op=mybir.AluOpType.mult)
                            pT = p_pool.tile([P, SKB], BF16, tag="p")
                            nc.scalar.activation(
                                pT[:], sm[:],
                                mybir.ActivationFunctionType.Exp,
                                scale=scale)
                            for j2 in range(SKB // P):
                                nc.tensor.matmul(
                                    o_ps[j2][:],
                                    pT[:, j2 * P:(j2 + 1) * P],
                                    v_sb[pkc][:],
                                    start=(pkc == 0),
                                    stop=(pkc == NKC - 1))
                        prev = (kc, sp) if sp is not None else None

                    # epilogue: normalize by the ones-column rowsum and store
                    for j2 in range(SKB // P):
                        osb = o_sb_pool.tile([P, HP1], F32, tag="osb")
                        nc.scalar.copy(osb[:], o_ps[j2][:])
                        den = den_pool.tile([P, 1], F32, tag="den")
                        nc.vector.reciprocal(den[:], osb[:, H:HP1])
                        oout = o_out_pool.tile([P, H], F32, tag="oout")
                        nc.vector.tensor_scalar_mul(
                            oout[:], osb[:, 0:H], den[:])
                        r0 = sqb * SKB + j2 * P
                        nc.sync.dma_start(out=out[r0:r0 + P, :], in_=oout[:])

    nc.compile()
    return nc


_COMPILED = None

# test-harness knobs (ignored in normal use)
TRACE = False
LAST_RESULT = None


def _get_compiled():
    global _COMPILED
    if _COMPILED is None:
        _COMPILED = build_attention_nc(num_devices=N_CORES)
    return _COMPILED


def _block_xT(xT):
    """[DV, W] f32 -> [W//SKB, P, DC, SKB] bf16 with
    blocks[wb, p, dc, w] = xT[dc*P + p, wb*SKB + w]."""
    W = xT.shape[1]
    b = xT.reshape(DC, P, W // SKB, SKB).transpose(2, 1, 0, 3)
    return np.ascontiguousarray(b.astype(ml_dtypes.bfloat16))


def _block_w(w):
    """[H, DV] f32 -> [P, DC, H] bf16 with blocks[p, dc, h] = w[h, dc*P+p]."""
    b = w.T.reshape(DC, P, H).transpose(1, 0, 2)
    return np.ascontiguousarray(b.astype(ml_dtypes.bfloat16))


def _block_maskT(maskT):
    """[S, SQL] -> [NSQB, P, NKC, SKB] fp8 with
    blocks[sqb, p, c, w] = maskT[c*P + p, sqb*SKB + w]."""
    b = maskT.reshape(NKC, P, NSQB, SKB).transpose(2, 1, 0, 3)
    return np.ascontiguousarray(b.astype(ml_dtypes.float8_e4m3))


def kernel(x_q, x_k, x_v, mask, wq_w, wq_b, wk_w, wk_b, wv_w, wv_b):
    """Full inputs in, full output out. Shards across 8 NeuronCores."""
    nc = _get_compiled()

    x_q = np.asarray(x_q, dtype=np.float32)
    x_k = np.asarray(x_k, dtype=np.float32)
    x_v = np.asarray(x_v, dtype=np.float32)
    mask = np.asarray(mask)

    wqb = _block_w(np.asarray(wq_w, dtype=np.float32))
    wkb = _block_w(np.asarray(wk_w, dtype=np.float32))
    wvb = _block_w(np.asarray(wv_w, dtype=np.float32))

    in_maps = []
    for c in range(N_CORES):
        b, half = divmod(c, CORES_PER_BATCH)
        q0 = half * SQL
        xqT = x_q[b][q0:q0 + SQL].T            # [DV, SQL]
        xkT = x_k[b].T                         # [DV, S]
        xvT = x_v[b].T
        maskT = mask[b][q0:q0 + SQL].T         # [S(k), SQL(q)]
        in_maps.append({
            "x_q": _block_xT(xqT),
            "x_k": _block_xT(xkT),
            "x_v": _block_xT(xvT),
            "mT": _block_maskT(maskT),
            "wq": wqb,
            "wk": wkb,
            "wv": wvb,
        })

    global LAST_RESULT
    res = run_bass_kernel_spmd(nc, in_maps, core_ids=list(range(N_CORES)),
                               trace=TRACE)
    LAST_RESULT = res
    outs = res.results

    full = np.empty((B, S, H), dtype=np.float32)
    for c in range(N_CORES):
        b, half = divmod(c, CORES_PER_BATCH)
        q0 = half * SQL
        full[b, q0:q0 + SQL] = outs[c]["out"]
    return full
